# revision 4
# baseline (speedup 1.0000x reference)
"""GNN message-passing (GAT-like bipartite) Trainium2 kernel, 8 NeuronCores.

Math (see reference):
  s_msg = x_source @ w_s ; t_msg = x_target @ w_t
  p_s = s_msg @ a1 ; p_t = t_msg @ a2
  ev_e = LReLU(p_s[col_e] + p_t[row_e])
  rowsum_t[i] = sum_{row=i} ev ; rowsum_s[j] = sum_{col=j} ev
  Yt[i] = (1/rowsum_t[i]) * sum_{row=i} n_e*ev_e*s_msg[col_e]
  Ys[j] = (1/rowsum_s[j]) * sum_{col=j} n_e*ev_e*t_msg[row_e]

Key identity: the normalization divides are per-OUTPUT-row, so they hoist out
of the edge loop.  Per-edge weight u_e = n_e*ev_e is used for both directions,
and rowsums are accumulated as an extra output column via a host-provided
1/n_e rhs value (u_e * (1/n_e) = ev_e).

Sharding: edges partitioned by source block (12500 sources/core).  Each core:
 - computes s_msg for its source shard and the full t_msg (matmuls on PE)
 - computes u_e on DVE from host-prepared spt_e = p_s[col]+p_t[row] streams
 - gathers s_msg rows per edge (dma_gather), aggregates per 128-target tile
   with one-hot matmuls into PSUM:  psum[:, :256] += (oh*u).T @ G,
   psum[:, 256] += (oh*u).T @ ninv  (= partial rowsum_t)
 - symmetric pass for Ys over its own source tiles (gathers t_msg rows)
Host: sums Yt partials over cores, applies the rowsum divides, re-assembles.
p_s/p_t (two tiny matvecs) are computed host-side to feed the spt streams.
"""

import os
import numpy as np

import concourse.bacc as bacc
import concourse.mybir as mybir
import concourse.tile as tile
from concourse.bass_utils import run_bass_kernel_spmd
from contextlib import ExitStack

P = 128
N_CORES = 8
LAST_EXEC_NS = None

OH_DT = mybir.dt.bfloat16   # one-hot blob storage dtype
OH_NP = np.dtype("uint16")  # shipped as raw uint16 bits of bf16


def _bf16_one(shape):
    # bf16 1.0 == 0x3F80
    a = np.zeros(shape, dtype=np.uint16)
    return a


class Cfg:
    def __init__(self, n_s, n_t, d, n_cores):
        assert n_s % n_cores == 0
        self.n_s, self.n_t, self.d = n_s, n_t, d
        self.sh = n_s // n_cores                    # sources per core
        self.sh_pad = ((self.sh + P - 1) // P) * P  # padded shard rows
        self.nt_pad = ((n_t + P - 1) // P) * P      # padded target rows
        self.s_tiles = self.sh_pad // P
        self.t_tiles = self.nt_pad // P
        self.kc = d // P                            # k-chunks for matmul


def _pack_idx(idx_flat, n_chunk_cols):
    """int16 gather indices -> [128, cols] wrapped in 16 partitions,
    replicated to the 8 q7 cores. idx i of a chunk at [i%16, base+i//16]."""
    out = np.zeros((P, n_chunk_cols), dtype=np.int16)
    n = idx_flat.shape[0]
    assert n % 16 == 0
    w = idx_flat.reshape(n // 16, 16).T  # [16, n/16]
    for c in range(8):
        out[16 * c:16 * c + 16, : n // 16] = w
    return out


def _build_side(cfg, rows, cols, n_vals, p_s_sh, p_t, n_tiles, G, tile_of, local_in_tile, gather_idx):
    """Build per-slot streams for one aggregation direction.

    rows/cols/n_vals: this core's edges (any order).
    tile_of: per-edge output tile index (len E_c)
    local_in_tile: per-edge one-hot column (0..127) within its tile
    gather_idx: per-edge gather row into the (<=32768-row) table
    Returns dict of arrays.
    """
    e_c = rows.shape[0]
    n_slots = n_tiles * G * P
    order = np.argsort(tile_of, kind="stable")
    t_sorted = tile_of[order]
    counts = np.bincount(t_sorted, minlength=n_tiles)
    assert counts.max() <= G * P, (counts.max(), G * P)
    starts = np.zeros(n_tiles, dtype=np.int64)
    starts[1:] = np.cumsum(counts)[:-1]
    rank = np.arange(e_c, dtype=np.int64) - starts[t_sorted]
    slot = t_sorted.astype(np.int64) * (G * P) + rank  # slot per sorted edge

    gidx = np.zeros(n_slots, dtype=np.int16)
    spt = np.zeros(n_slots, dtype=np.float32)
    nv = np.zeros(n_slots, dtype=np.float32)
    ninv = np.zeros(n_slots, dtype=np.float32)
    oh_col = np.full(n_slots, -1, dtype=np.int32)

    eo_rows = rows[order]
    eo_cols = cols[order]
    eo_n = n_vals[order]
    gidx[slot] = gather_idx[order].astype(np.int16)
    spt[slot] = p_s_sh[eo_cols] + p_t[eo_rows]
    nv[slot] = eo_n
    with np.errstate(divide="ignore"):
        ninv_e = np.where(eo_n != 0, 1.0 / np.maximum(eo_n, 1e-38), 0.0).astype(np.float32)
    ninv[slot] = ninv_e
    oh_col[slot] = local_in_tile[order]

    # one-hot blob: [128, n_groups*128] bf16-bits; partition = slot%128
    n_groups = n_tiles * G
    oh = np.zeros((n_groups * P, P), dtype=np.uint16)
    valid = oh_col >= 0
    oh[np.nonzero(valid)[0], oh_col[valid]] = 0x3F80  # bf16 1.0
    oh = oh.reshape(n_groups, P, P).transpose(1, 0, 2).reshape(P, n_groups * P)

    # scalar streams as [128, n_groups]: value for slot s at [s%128, s//128]
    def wrap(a):
        return a.reshape(n_groups, P).T.copy()

    # gather idx array: per tile, chunks of (1024, rem) -> per tile idx slab
    # [128, G*8] at col offset tile*G*8 (chunk A cols 0..63, chunk B 64..)
    cols_per_tile = G * 8
    idx_arr = np.zeros((P, n_tiles * cols_per_tile), dtype=np.int16)
    for t in range(n_tiles):
        sl = gidx[t * G * P:(t + 1) * G * P]
        for k0 in range(0, G * P, 1024):
            ck = min(1024, G * P - k0)
            a = _pack_idx(sl[k0:k0 + ck], ck // 16)
            co = t * cols_per_tile + (k0 // 1024) * 64
            idx_arr[:, co: co + ck // 16] = a
    return {
        "idx": idx_arr, "oh": oh, "spt": wrap(spt), "n": wrap(nv),
        "ninv": wrap(ninv),
    }


def _build_program(cfg, Gt, Gs):
    d = cfg.d
    nc = bacc.Bacc("TRN2", target_bir_lowering=False, debug=False,
                   num_devices=N_CORES)
    f32 = mybir.dt.float32

    def din(name, shape, dt=f32):
        return nc.dram_tensor(name, shape, dt, kind="ExternalInput").ap()

    def dout(name, shape, dt=f32):
        return nc.dram_tensor(name, shape, dt, kind="ExternalOutput").ap()

    xsT = din("xsT", [P, cfg.kc, cfg.sh_pad])          # x_source shard, feat-major
    xtT = din("xtT", [P, cfg.kc, cfg.nt_pad])          # x_target full, feat-major
    w_s = din("w_s", [P, cfg.kc, d])
    w_t = din("w_t", [P, cfg.kc, d])
    yt_idx = din("yt_idx", [P, cfg.t_tiles * Gt * 8], mybir.dt.int16)
    ys_idx = din("ys_idx", [P, cfg.s_tiles * Gs * 8], mybir.dt.int16)
    yt_oh = din("yt_oh", [P, cfg.t_tiles * Gt * P], OH_DT)
    ys_oh = din("ys_oh", [P, cfg.s_tiles * Gs * P], OH_DT)
    yt_spt = din("yt_spt", [P, cfg.t_tiles * Gt])
    yt_n = din("yt_n", [P, cfg.t_tiles * Gt])
    yt_ninv = din("yt_ninv", [P, cfg.t_tiles * Gt])
    ys_spt = din("ys_spt", [P, cfg.s_tiles * Gs])
    ys_n = din("ys_n", [P, cfg.s_tiles * Gs])
    ys_ninv = din("ys_ninv", [P, cfg.s_tiles * Gs])

    s_msg = nc.dram_tensor("s_msg", [cfg.sh_pad, d], f32).ap()
    t_msg = nc.dram_tensor("t_msg", [cfg.nt_pad, d], f32).ap()
    yt_part = dout("yt_part", [cfg.nt_pad, d + 1])
    ys_part = dout("ys_part", [cfg.sh_pad, d + 1])

    with tile.TileContext(nc) as tc, ExitStack() as ctx:
        const = ctx.enter_context(tc.tile_pool(name="const", bufs=1))
        xpool = ctx.enter_context(tc.tile_pool(name="xp", bufs=2))
        mpool = ctx.enter_context(tc.tile_pool(name="mp", bufs=2))
        psum = ctx.enter_context(tc.tile_pool(name="ps", bufs=4, space="PSUM"))
        gpool = ctx.enter_context(tc.tile_pool(name="gp", bufs=2))
        ohpool = ctx.enter_context(tc.tile_pool(name="ohp", bufs=2))
        wpool = ctx.enter_context(tc.tile_pool(name="wp", bufs=1))
        opool = ctx.enter_context(tc.tile_pool(name="op", bufs=2))

        # ---- phase 1: s_msg / t_msg matmuls ----
        ws_t = const.tile([P, cfg.kc, d], f32)
        wt_t = const.tile([P, cfg.kc, d], f32)
        nc.sync.dma_start(ws_t[:], w_s[:])
        nc.sync.dma_start(wt_t[:], w_t[:])

        BT = 8  # tiles per streaming chunk

        def mm_phase(xT_hbm, n_tiles, out_hbm):
            n_chunks = (n_tiles + BT - 1) // BT
            for jj in range(n_chunks):
                t0 = jj * BT
                nb = min(BT, n_tiles - t0)
                xt_t = xpool.tile([P, cfg.kc, BT * P], f32, tag="xs")
                nc.sync.dma_start(
                    xt_t[:, :, :nb * P],
                    xT_hbm[:, :, t0 * P:(t0 + nb) * P])
                stage = mpool.tile([P, BT, d], f32, tag="stage")
                for j in range(nb):
                    pst = psum.tile([P, d], f32, tag="mm")
                    for k in range(cfg.kc):
                        nc.tensor.matmul(
                            pst[:],
                            xt_t[:, k, (j * P):(j + 1) * P],
                            ws_t[:, k, :] if out_hbm is s_msg else wt_t[:, k, :],
                            start=(k == 0), stop=(k == cfg.kc - 1))
                    nc.scalar.copy(stage[:, j, :], pst[:])
                nc.sync.dma_start(
                    out_hbm[t0 * P:(t0 + nb) * P, :].rearrange(
                        "(g p) d -> p g d", p=P),
                    stage[:, :nb, :])

        mm_phase(xsT, cfg.s_tiles, s_msg)
        mm_phase(xtT, cfg.t_tiles, t_msg)

        # ---- upfront per-slot scalars: u = n * lrelu(spt) ----
        def scalars(spt_hbm, n_hbm, ninv_hbm, ncols):
            spt_t = const.tile([P, ncols], f32, tag=f"spt{ncols}{spt_hbm.name}")
            n_t = const.tile([P, ncols], f32, tag=f"n{ncols}{n_hbm.name}")
            ninv_t = const.tile([P, ncols], f32, tag=f"ninv{ncols}{ninv_hbm.name}")
            nc.sync.dma_start(spt_t[:], spt_hbm[:])
            nc.sync.dma_start(n_t[:], n_hbm[:])
            nc.sync.dma_start(ninv_t[:], ninv_hbm[:])
            pos = const.tile([P, ncols], f32, tag=f"pos{ncols}{spt_hbm.name}")
            # pos = max(spt, 0); spt' = min(spt,0)*0.2 ; ev = pos+spt' ; u = ev*n
            nc.vector.tensor_scalar(pos[:], spt_t[:], 0.0, None, mybir.AluOpType.max)
            nc.vector.tensor_scalar(spt_t[:], spt_t[:], 0.0, 0.2,
                                    mybir.AluOpType.min, mybir.AluOpType.mult)
            nc.vector.tensor_add(spt_t[:], spt_t[:], pos[:])
            nc.vector.tensor_mul(spt_t[:], spt_t[:], n_t[:])
            return spt_t, ninv_t  # u, ninv

        u_yt, ninv_yt = scalars(yt_spt, yt_n, yt_ninv, cfg.t_tiles * Gt)
        u_ys, ninv_ys = scalars(ys_spt, ys_n, ys_ninv, cfg.s_tiles * Gs)

        # ---- aggregation passes ----
        def agg_phase(n_tiles, G, idx_hbm, oh_hbm, u_t, ninv_t, table_hbm,
                      out_hbm, idx_t):
            cpt = G * 8
            for t in range(n_tiles):
                gbuf = gpool.tile([P, G, d], f32, tag="gbuf")
                for k0 in range(0, G * P, 1024):
                    ck = min(1024, G * P - k0)
                    co = t * cpt + (k0 // 1024) * 64
                    nc.gpsimd.dma_gather(
                        gbuf[:, k0 // P:(k0 + ck) // P, :], table_hbm[:],
                        idx_t[:, co: co + ck // 16],
                        num_idxs=ck, num_idxs_reg=ck, elem_size=d)
                oh_t = ohpool.tile([P, G * P], OH_DT, tag="oh")
                nc.sync.dma_start(oh_t[:], oh_hbm[:, t * G * P:(t + 1) * G * P])
                # weighted one-hot: lhsT_w[:, g*128:(g+1)*128] = oh * u[:, tile*G+g]
                lw = wpool.tile([P, G * P], f32, tag="lw")
                for g in range(G):
                    nc.vector.tensor_scalar_mul(
                        lw[:, g * P:(g + 1) * P], oh_t[:, g * P:(g + 1) * P],
                        u_t[:, t * G + g: t * G + g + 1])
                pst = psum.tile([P, d + 1], f32, tag="agg")
                for g in range(G):
                    nc.tensor.matmul(
                        pst[:, :d], lw[:, g * P:(g + 1) * P],
                        gbuf[:, g, :],
                        start=(g == 0), stop=(g == G - 1))
                for g in range(G):
                    nc.tensor.matmul(
                        pst[:, d:d + 1], lw[:, g * P:(g + 1) * P],
                        ninv_t[:, t * G + g: t * G + g + 1],
                        start=(g == 0), stop=(g == G - 1))
                ot = opool.tile([P, d + 1], f32, tag="ot")
                nc.scalar.copy(ot[:], pst[:])
                nc.sync.dma_start(out_hbm[t * P:(t + 1) * P, :], ot[:])

        yt_idx_t = const.tile([P, cfg.t_tiles * Gt * 8], mybir.dt.int16)
        nc.sync.dma_start(yt_idx_t[:], yt_idx[:])
        ys_idx_t = const.tile([P, cfg.s_tiles * Gs * 8], mybir.dt.int16)
        nc.sync.dma_start(ys_idx_t[:], ys_idx[:])

        agg_phase(cfg.t_tiles, Gt, yt_idx, yt_oh, u_yt, ninv_yt, s_msg,
                  yt_part, yt_idx_t)
        agg_phase(cfg.s_tiles, Gs, ys_idx, ys_oh, u_ys, ninv_ys, t_msg,
                  ys_part, ys_idx_t)

    nc.compile()
    return nc


def _prep(cfg, x_source, x_target, n_vals, w_s, w_t, att_weight,
          edge_row, edge_col):
    d = cfg.d
    a1, a2 = att_weight[:d], att_weight[d:]
    p_s = (x_source @ (w_s @ a1)).astype(np.float32)
    p_t = (x_target @ (w_t @ a2)).astype(np.float32)

    core = edge_col // cfg.sh
    in_maps = []
    metas = []
    # global max counts decide Gt/Gs (uniform across cores)
    Gt = Gs = 0
    percore = []
    for c in range(N_CORES):
        m = core == c
        r, cl, nv = edge_row[m], edge_col[m] - c * cfg.sh, n_vals[m]
        tt = r // P
        ts = cl // P
        ct = np.bincount(tt, minlength=cfg.t_tiles).max()
        cs = np.bincount(ts, minlength=cfg.s_tiles).max()
        Gt = max(Gt, (int(ct) + P - 1) // P)
        Gs = max(Gs, (int(cs) + P - 1) // P)
        percore.append((r, cl, nv))

    kc = cfg.kc
    wsr = np.ascontiguousarray(
        w_s.reshape(kc, P, d).transpose(1, 0, 2)).astype(np.float32)
    wtr = np.ascontiguousarray(
        w_t.reshape(kc, P, d).transpose(1, 0, 2)).astype(np.float32)
    xtT = np.zeros((P, kc, cfg.nt_pad), dtype=np.float32)
    xtT[:, :, :cfg.n_t] = x_target.T.reshape(kc, P, cfg.n_t).transpose(1, 0, 2)

    for c in range(N_CORES):
        r, cl, nv = percore[c]
        xs = x_source[c * cfg.sh:(c + 1) * cfg.sh]
        xsT = np.zeros((P, kc, cfg.sh_pad), dtype=np.float32)
        xsT[:, :, :cfg.sh] = xs.T.reshape(kc, P, cfg.sh).transpose(1, 0, 2)
        p_s_sh = p_s[c * cfg.sh:(c + 1) * cfg.sh]

        yt = _build_side(cfg, r, cl, nv, p_s_sh, p_t, cfg.t_tiles, Gt,
                         tile_of=r // P, local_in_tile=r % P, gather_idx=cl)
        ys = _build_side(cfg, r, cl, nv, p_s_sh, p_t, cfg.s_tiles, Gs,
                         tile_of=cl // P, local_in_tile=cl % P, gather_idx=r)
        in_maps.append({
            "xsT": xsT, "xtT": xtT, "w_s": wsr, "w_t": wtr,
            "yt_idx": yt["idx"], "ys_idx": ys["idx"],
            "yt_oh": yt["oh"].view(np.uint16), "ys_oh": ys["oh"].view(np.uint16),
            "yt_spt": yt["spt"], "yt_n": yt["n"], "yt_ninv": yt["ninv"],
            "ys_spt": ys["spt"], "ys_n": ys["n"], "ys_ninv": ys["ninv"],
        })
        metas.append((r, cl, nv))
    return in_maps, metas, Gt, Gs, p_s, p_t


def _run(cfg, x_source, x_target, n_vals, w_s, w_t, att_weight,
         edge_row, edge_col):
    global LAST_EXEC_NS
    in_maps, metas, Gt, Gs, p_s, p_t = _prep(
        cfg, x_source, x_target, n_vals, w_s, w_t, att_weight,
        edge_row, edge_col)
    nc = _build_program(cfg, Gt, Gs)

    trace = bool(os.environ.get("BASS_TRACE"))
    if trace:
        try:
            from trn_agent_boot.trn_boot import _ntff_profile_via_ctypes
            from antenv.axon_hooks import set_axon_ntff_profile_hook
            set_axon_ntff_profile_hook(
                _ntff_profile_via_ctypes("/opt/axon/libaxon_pjrt.so"))
        except Exception as e:
            print("profile hook registration failed:", e)
    r = run_bass_kernel_spmd(nc, in_maps, list(range(N_CORES)), trace=trace)
    LAST_EXEC_NS = r.exec_time_ns

    d = cfg.d
    yt_acc = np.zeros((cfg.nt_pad, d), dtype=np.float32)
    rs_t = np.zeros(cfg.nt_pad, dtype=np.float32)
    ys_full = np.zeros((cfg.n_s, d), dtype=np.float32)
    rs_s = np.zeros(cfg.n_s, dtype=np.float32)
    for c in range(N_CORES):
        ytp = r.results[c]["yt_part"]
        yt_acc += ytp[:, :d]
        rs_t += ytp[:, d]
        ysp = r.results[c]["ys_part"]
        ys_full[c * cfg.sh:(c + 1) * cfg.sh] = ysp[:cfg.sh, :d]
        rs_s[c * cfg.sh:(c + 1) * cfg.sh] = ysp[:cfg.sh, d]

    # host correction for n_vals == 0 edges (their ev is missing from the
    # device rowsums since rowsum column is u*ninv with u = n*ev = 0)
    zmask = n_vals == 0
    if zmask.any():
        ev0 = p_s[edge_col[zmask]] + p_t[edge_row[zmask]]
        ev0 = np.where(ev0 >= 0, ev0, 0.2 * ev0)
        np.add.at(rs_t, edge_row[zmask], ev0)
        np.add.at(rs_s, edge_col[zmask], ev0)

    with np.errstate(divide="ignore", invalid="ignore"):
        yt = np.where(rs_t[:cfg.n_t, None] != 0,
                      yt_acc[:cfg.n_t] / rs_t[:cfg.n_t, None], 0.0)
        ys = np.where(rs_s[:, None] != 0, ys_full / rs_s[:, None], 0.0)
    return ys.astype(np.float32), yt.astype(np.float32)


def kernel(x_source, x_target, n_vals, w_s, w_t, att_weight,
           edge_row, edge_col):
    cfg = Cfg(100000, 25000, 256, N_CORES)
    x_source = np.asarray(x_source, dtype=np.float32)
    x_target = np.asarray(x_target, dtype=np.float32)
    n_vals = np.asarray(n_vals, dtype=np.float32)
    w_s = np.asarray(w_s, dtype=np.float32)
    w_t = np.asarray(w_t, dtype=np.float32)
    att_weight = np.asarray(att_weight, dtype=np.float32)
    edge_row = np.asarray(edge_row).astype(np.int64)
    edge_col = np.asarray(edge_col).astype(np.int64)
    return _run(cfg, x_source, x_target, n_vals, w_s, w_t, att_weight,
                edge_row, edge_col)
